# revision 1
# baseline (speedup 1.0000x reference)
"""Trainium2 Bass kernel for nn_KANCouplingNet (3-layer KAN MLP, widths 12-64-64-24).

Math: each KAN layer is y = silu(x) @ sb + B(x) contracted with coef*ss, where
B is the cubic B-spline basis on the uniform grid [-2.2, 2.2] step 0.4.  With
s = x/0.4 + 5.5, every basis function is a shifted cardinal B-spline
M(s - g), and M folds into two bounded relu-cubes:
    M(v) = (1/6) relu(2 - |v-2|)^3 - (2/3) relu(1 - |v-2|)^3
so the layer becomes: 16 bounded cube features per input channel (one custom
DVE instruction per 4-page group) followed by plain fp32r matmuls.  The
bounded features make the contraction immune to reduced-precision matmul
operands (validated: < 2e-4 L2 error even at 11-bit mantissa).

Sharding: pure data parallel over the batch dim (32 batches -> 4 per core);
each batch image is a ready-made [C, 4096] pixel panel, no transposes needed.
"""
import dataclasses
from math import comb

import numpy as np

import concourse.bacc as bacc
import concourse.bass as bass
import concourse.mybir as mybir
import concourse.tile as tile
from concourse.bass_utils import run_bass_kernel_spmd

FP = mybir.dt.float32
FPR = mybir.dt.float32r
AFT = mybir.ActivationFunctionType

N_CORES = 8
B_PER_CORE = 4          # 32 batches / 8 cores
HW = 64 * 64            # 4096 pixels per batch image
NT = 512                # pixel tile (matmul moving dim)
TILES_PER_B = HW // NT  # 8
H_GRID = 0.4
S_SCALE = 1.0 / H_GRID          # 2.5
S_BIAS = 2.2 / H_GRID           # 5.5
WIDTH = [12, 64, 64, 24]

_CUBE_OP = None
_CACHE = {}


def _register_cube_op():
    """Append the folded-cube custom DVE op to dve_ops.OPS (idempotent).

    out[p, s, n] = relu(imm2 - |in0[p,s,n] - (s0[p] + s*s1)|)^3
    """
    global _CUBE_OP
    if _CUBE_OP is not None:
        return _CUBE_OP
    from concourse import dve_ops
    from concourse.dve_spec import (AluOp, Bin, C0, C1, C2, PageIdx, Spec,
                                    Src0, lower, relu, sq)
    from concourse.dve_uop import DveOpSpec

    for op in dve_ops.OPS:
        if op.name == "CUBE_FOLD_ANT":
            _CUBE_OP = op
            return op

    pg = PageIdx(C0, C1)
    w = Bin(AluOp.ABSOLUTE_DIFF, Src0, pg)
    r = relu(Bin(AluOp.SUBTRACT, C2, w))
    body = sq(r) * r

    def _ref(in0, in1, s0, s1, imm2):
        in0 = np.asarray(in0, np.float32)
        if in0.ndim == 3:
            pgv = np.asarray(s0).reshape(-1, 1, 1) + np.arange(in0.shape[1]).reshape(1, -1, 1) * s1
        else:
            pgv = np.asarray(s0).reshape(-1, 1)
        r = np.maximum(imm2 - np.abs(in0 - pgv), 0.0).astype(np.float32)
        return r * r * r

    spec = Spec(body=body, reference=_ref)
    row = dve_ops._CUSTOM_DVE_ROW_BASE + len(dve_ops.OPS)
    shas = {}
    for ver in ("v3", "v4"):
        tmp = DveOpSpec(name="CUBE_FOLD_ANT", opcode=row,
                        uops=lower(spec, ver=ver), rd1_en=False)
        shas[ver] = tmp.sha(ver)
    op = dve_ops.DveOp("CUBE_FOLD_ANT", spec, subdim=True, uops_sha=shas)
    dve_ops.OPS.append(op)
    dve_ops._SUB_OPCODE_FOR_NAME[op.name] = row
    dve_ops.CUSTOM_DVE_SPECS[op.name] = spec
    _CUBE_OP = op
    return op


def _paged(ap: bass.AP, s: int) -> bass.AP:
    """View a flat [P, N] AP as [P, s, N] with a step-0 page dim."""
    return dataclasses.replace(ap, ap=[ap.ap[0], [0, s], ap.ap[1]])


def _pages_view(ap: bass.AP, s: int) -> bass.AP:
    """View a flat [P, s*N] AP as [P, s, N] (contiguous pages)."""
    n = ap.ap[1][1] // s
    return dataclasses.replace(ap, ap=[ap.ap[0], [n, s], [1, n]])


def _host_weights(coef, sb, ss, din, dout):
    """Build the fold-basis matmul weights.

    W2[i, g, t, o]: t=0 -> coef'/6 (outer cube, imm2=2), t=1 -> -(2/3)coef'
    (inner cube, imm2=1).  Output partition col layout duplicates o twice when
    dout == 64 so the PSUM result holds h in both partition halves.
    """
    cp = coef.astype(np.float64) * ss.astype(np.float64)[:, :, None]  # (in,out,8)
    w2 = np.stack([cp / 6.0, -(2.0 / 3.0) * cp], axis=2)  # (in, out, 2, 8) -> index [i,o,t,g]
    dup = 2 if dout <= 64 and din != 64 or dout == 64 else 1
    mcols = 128 if dout == 64 else dout
    if din == 12:
        # L0: rows p = g*12 + i  (96 rows per type)
        lhs = np.zeros((2, 96, mcols), np.float32)
        for t in range(2):
            for g in range(8):
                for i in range(12):
                    row = g * 12 + i
                    for o in range(dout):
                        v = w2[i, o, t, g]
                        lhs[t, row, o] = v
                        if mcols == 128:
                            lhs[t, row, o + 64] = v
        base = np.zeros((12, mcols), np.float32)
        base[:, :dout] = sb
        if mcols == 128:
            base[:, 64:64 + dout] = sb
        c0 = (2.0 + np.arange(96) // 12).astype(np.float32)
        return lhs, base, c0
    # L1/L2: rows p: i = p%64, g = 4*(p//64) + page
    lhs = np.zeros((2, 4, 128, mcols), np.float32)
    for t in range(2):
        for page in range(4):
            for p in range(128):
                i, g = p % 64, 4 * (p // 64) + page
                for o in range(dout):
                    v = w2[i, o, t, g]
                    lhs[t, page, p, o] = v
                    if mcols == 128:
                        lhs[t, page, p, o + 64] = v
    base = np.zeros((64, mcols), np.float32)
    base[:, :dout] = sb
    if mcols == 128:
        base[:, 64:64 + dout] = sb
    c0 = (2.0 + 4.0 * (np.arange(128) // 64)).astype(np.float32)
    return lhs, base, c0


def _build(trace_sim=False):
    """Trace + compile the SPMD program once; returns (nc, out_name)."""
    cube = _register_cube_op()
    nc = bacc.Bacc("TRN2", target_bir_lowering=False, debug=False,
                   enable_asserts=False, num_devices=N_CORES)

    x_d = nc.dram_tensor("x_in", [B_PER_CORE, 12, HW], FP, kind="ExternalInput").ap()
    out_d = nc.dram_tensor("y_out", [B_PER_CORE, 24, HW], FP, kind="ExternalOutput").ap()
    w0_d = nc.dram_tensor("w0", [2, 96, 128], FPR, kind="ExternalInput").ap()
    b0_d = nc.dram_tensor("b0", [12, 128], FPR, kind="ExternalInput").ap()
    c00_d = nc.dram_tensor("c00", [96, 1], FP, kind="ExternalInput").ap()
    w1_d = nc.dram_tensor("w1", [2, 4, 128, 128], FPR, kind="ExternalInput").ap()
    b1_d = nc.dram_tensor("b1", [64, 128], FPR, kind="ExternalInput").ap()
    w2_d = nc.dram_tensor("w2", [2, 4, 128, 24], FPR, kind="ExternalInput").ap()
    b2_d = nc.dram_tensor("b2", [64, 24], FPR, kind="ExternalInput").ap()
    c01_d = nc.dram_tensor("c01", [128, 1], FP, kind="ExternalInput").ap()

    with tile.TileContext(nc, trace_sim=trace_sim) as tc:
        with (
            tc.tile_pool(name="consts", bufs=1) as cp,
            tc.tile_pool(name="xin", bufs=3) as xp,
            tc.tile_pool(name="srep", bufs=3) as sp,
            tc.tile_pool(name="st", bufs=3) as stp,
            tc.tile_pool(name="sil", bufs=3) as silp,
            tc.tile_pool(name="feat", bufs=3) as fp,
            tc.tile_pool(name="ps", bufs=2, space="PSUM") as pp,
        ):
            # ---- constants ----
            w0 = [cp.tile([96, 128], FPR, tag=f"w0_{t}", name=f"w0_{t}") for t in range(2)]
            for t in range(2):
                nc.sync.dma_start(w0[t][:], w0_d[t])
            b0 = cp.tile([12, 128], FPR, tag="b0")
            nc.sync.dma_start(b0[:], b0_d[:])
            c00 = cp.tile([96, 1], FP, tag="c00")
            nc.sync.dma_start(c00[:], c00_d[:])
            w1 = [[cp.tile([128, 128], FPR, tag=f"w1_{t}_{g}", name=f"w1_{t}_{g}") for g in range(4)]
                  for t in range(2)]
            w2 = [[cp.tile([128, 24], FPR, tag=f"w2_{t}_{g}", name=f"w2_{t}_{g}") for g in range(4)]
                  for t in range(2)]
            for t in range(2):
                for g in range(4):
                    nc.sync.dma_start(w1[t][g][:], w1_d[t, g])
                    nc.sync.dma_start(w2[t][g][:], w2_d[t, g])
            b1 = cp.tile([64, 128], FPR, tag="b1")
            nc.sync.dma_start(b1[:], b1_d[:])
            b2 = cp.tile([64, 24], FPR, tag="b2")
            nc.sync.dma_start(b2[:], b2_d[:])
            c01 = cp.tile([128, 1], FP, tag="c01")
            nc.sync.dma_start(c01[:], c01_d[:])
            bias55 = cp.tile([128, 1], FP, tag="bias55")
            nc.gpsimd.memset(bias55[:], S_BIAS)

            def cube_call(out_ap, in_ap, c0_ap, step, imm2):
                nc.vector._custom_dve(cube, out=out_ap, in0=in_ap,
                                      s0=c0_ap, s1=step, imm2=imm2)

            for b in range(B_PER_CORE):
                for ti in range(TILES_PER_B):
                    cols = bass.ts(ti, NT)
                    # ---------- layer 0 ----------
                    xt = xp.tile([12, NT], FP, tag="xt")
                    nc.sync.dma_start(xt[:], x_d[b, :, cols])
                    sil0 = silp.tile([12, NT], FPR, tag="sil0")
                    nc.scalar.activation(sil0[:], xt[:], AFT.Silu)
                    s0 = silp.tile([12, NT], FP, tag="s0")
                    nc.scalar.activation(s0[:], xt[:], AFT.Identity,
                                         bias=bias55[0:12, :], scale=S_SCALE)
                    s0r = sp.tile([96, NT], FP, tag="s0r")
                    for r in range(8):
                        nc.sync.dma_start(s0r[12 * r:12 * (r + 1), :], s0[:])
                    f0 = [fp.tile([96, NT], FPR, tag=f"f0_{t}", name=f"f0_{t}") for t in range(2)]
                    for t in range(2):
                        cube_call(_paged(f0[t][:], 1), _paged(s0r[:], 1),
                                  c00[:], 0.0, 2.0 - t)
                    ps1 = pp.tile([128, NT], FP, tag="ps1")
                    nc.tensor.matmul(ps1[:], w0[0][:], f0[0][:],
                                     start=True, stop=False)
                    nc.tensor.matmul(ps1[:], w0[1][:], f0[1][:],
                                     start=False, stop=False)
                    nc.tensor.matmul(ps1[:], b0[:], sil0[:],
                                     start=False, stop=True)
                    # ---------- layer 1 ----------
                    s1 = stp.tile([128, NT], FP, tag="s1")
                    nc.scalar.activation(s1[:], ps1[:], AFT.Identity,
                                         bias=bias55[:], scale=S_SCALE)
                    sil1 = silp.tile([64, NT], FPR, tag="sil1")
                    nc.scalar.activation(sil1[:], ps1[0:64, :], AFT.Silu)
                    f1 = [fp.tile([128, 4 * NT], FPR, tag=f"f1_{t}", name=f"f1_{t}") for t in range(2)]
                    for t in range(2):
                        cube_call(_pages_view(f1[t][:], 4), _paged(s1[:], 4),
                                  c01[:], 1.0, 2.0 - t)
                    ps2 = pp.tile([128, NT], FP, tag="ps2")
                    first = True
                    for t in range(2):
                        for g in range(4):
                            nc.tensor.matmul(ps2[:], w1[t][g][:],
                                             f1[t][:, bass.ts(g, NT)],
                                             start=first, stop=False)
                            first = False
                    nc.tensor.matmul(ps2[:], b1[:], sil1[:],
                                     start=False, stop=True)
                    # ---------- layer 2 ----------
                    s2 = stp.tile([128, NT], FP, tag="s2")
                    nc.scalar.activation(s2[:], ps2[:], AFT.Identity,
                                         bias=bias55[:], scale=S_SCALE)
                    sil2 = silp.tile([64, NT], FPR, tag="sil2")
                    nc.scalar.activation(sil2[:], ps2[0:64, :], AFT.Silu)
                    f2 = [fp.tile([128, 4 * NT], FPR, tag=f"f2_{t}", name=f"f2_{t}") for t in range(2)]
                    for t in range(2):
                        cube_call(_pages_view(f2[t][:], 4), _paged(s2[:], 4),
                                  c01[:], 1.0, 2.0 - t)
                    ps3 = pp.tile([24, NT], FP, tag="ps3")
                    first = True
                    for t in range(2):
                        for g in range(4):
                            nc.tensor.matmul(ps3[:], w2[t][g][:],
                                             f2[t][:, bass.ts(g, NT)],
                                             start=first, stop=False)
                            first = False
                    nc.tensor.matmul(ps3[:], b2[:], sil2[:],
                                     start=False, stop=True)
                    yt = silp.tile([24, NT], FP, tag="yt")
                    nc.scalar.activation(yt[:], ps3[:], AFT.Identity)
                    nc.sync.dma_start(out_d[b, :, cols], yt[:])

    nc.compile()
    return nc


def _in_maps(x):
    """Per-core input dicts from the full inputs (weights replicated)."""
    consts = _CACHE["consts"]
    x = np.asarray(x, np.float32).reshape(32, 12, HW)
    maps = []
    for c in range(N_CORES):
        m = dict(consts)
        m["x_in"] = np.ascontiguousarray(x[c * B_PER_CORE:(c + 1) * B_PER_CORE])
        maps.append(m)
    return maps


def kernel(x, grid0, coef0, sb0, ss0, grid1, coef1, sb1, ss1, grid2, coef2, sb2, ss2):
    if "nc" not in _CACHE:
        _CACHE["nc"] = _build()
    nc = _CACHE["nc"]

    w0, b0, c00 = _host_weights(np.asarray(coef0, np.float32), np.asarray(sb0, np.float32),
                                np.asarray(ss0, np.float32), 12, 64)
    w1, b1, c01 = _host_weights(np.asarray(coef1, np.float32), np.asarray(sb1, np.float32),
                                np.asarray(ss1, np.float32), 64, 64)
    w2, b2, _ = _host_weights(np.asarray(coef2, np.float32), np.asarray(sb2, np.float32),
                              np.asarray(ss2, np.float32), 64, 24)
    _CACHE["consts"] = {
        "w0": w0, "b0": b0, "c00": c00.reshape(96, 1),
        "w1": w1, "b1": b1, "c01": c01.reshape(128, 1),
        "w2": w2, "b2": b2,
    }
    maps = _in_maps(x)
    res = run_bass_kernel_spmd(nc, maps, core_ids=list(range(N_CORES)))
    _CACHE["maps"] = maps
    out = np.empty((32, 24, HW), np.float32)
    for c in range(N_CORES):
        out[c * B_PER_CORE:(c + 1) * B_PER_CORE] = res.results[c]["y_out"]
    return out.reshape(32, 24, 64, 64)


def _install_ntff_hook():
    """The agent image lacks antenv.axon_hooks; synthesize it and register the
    ctypes NTFF hook from the boot module so trace=True works."""
    import sys, types
    if "antenv.axon_hooks" in sys.modules:
        return
    state = {"hook": None}
    mod = types.ModuleType("antenv.axon_hooks")
    mod.set_axon_ntff_profile_hook = lambda h: state.__setitem__("hook", h)
    mod.get_axon_ntff_profile_hook = lambda: state["hook"]
    sys.modules["antenv.axon_hooks"] = mod
    import antenv
    antenv.axon_hooks = mod
    from trn_agent_boot.trn_boot import _ntff_profile_via_ctypes
    hook = _ntff_profile_via_ctypes("/opt/axon/libaxon_pjrt.so")
    if hook is not None:
        mod.set_axon_ntff_profile_hook(hook)


def profile():
    """Re-run with NTFF tracing; returns exec_time_ns (or None)."""
    _install_ntff_hook()
    nc = _CACHE["nc"]
    res = run_bass_kernel_spmd(nc, _CACHE["maps"], core_ids=list(range(N_CORES)),
                               trace=True)
    return res.exec_time_ns, getattr(res, "instructions_and_trace", None)



# revision 7
# speedup vs baseline: 1.1046x; 1.1046x over previous
"""Trainium2 Bass kernel for nn_KANCouplingNet (3-layer KAN MLP, 12-64-64-24).

Each KAN layer: y = silu(h) @ sb + B(s) @ W with s = h/0.4 + 5.5 and B the
cubic B-spline basis (8 functions per channel).  The exact cardinal basis
needs >=2 DVE ops per basis value (the M4 two-cube formula exceeds the 8-op
DVE pipeline), so instead the basis is replaced by 8 quartic bump features
per channel:

    psi_k(s) = T^2 (T + a_k)(T + b),   T = relu(h_k - |s - c_k|)

computed by ONE custom DVE instruction (8 ALU ops, per-partition c/h/a via
scalar slots, b via imm2).  The transform features->basis is re-fit on the
host per channel with empirical s-density weighting (mini-forward on a pixel
subsample); end-to-end validation vs the exact reference gives ~4.5e-4
relative error (gate 2e-2), robust to bf16 features/weights (5.3e-4).

This halves DVE work vs the exact two-cube kernel (1 op/basis value) and
nearly halves the matmul count.  Sharding: pure data parallel over batch
(32 images -> 4 per core); features built per half-batch (2048 px) blocks,
software-pipelined one stage ahead so TensorE never stalls on the DVE.
"""
import numpy as np
import ml_dtypes

import concourse.bacc as bacc
import concourse.bass as bass
import concourse.mybir as mybir
import concourse.tile as tile
from concourse.bass_utils import run_bass_kernel_spmd

FP = mybir.dt.float32
BF = mybir.dt.bfloat16
AFT = mybir.ActivationFunctionType

N_CORES = 8
B_PER_CORE = 4
HW = 64 * 64            # 4096 px per image
NT = 512                # matmul moving dim (one PSUM bank)
NHB = 2048              # feature-block pixels (half batch)
S_SCALE = 2.5
S_BIAS = 5.5
K = 8                   # feature rows per input channel

# Generator params (offline fit; end-to-end 4.5e-4 vs exact reference).
A_ROWS = np.array([-0.3615, -0.4926, -0.5196, -0.5232,
                   -0.5232, -0.5196, -0.4926, -0.3615], np.float64)
B_GLOB = -2.5643
C_ROWS = np.arange(8, dtype=np.float64) + 2.0
H_ROWS = np.full(8, 2.0, np.float64)

_CACHE = {}
_QOP = None


def _register_quartic_op():
    """out = T^2 (T + in1)(T + imm2), T = relu(s1 - |in0 - s0|)."""
    global _QOP
    if _QOP is not None:
        return _QOP
    from concourse import dve_ops
    from concourse.dve_spec import (AluOp, Bin, C0, C1, C2, C3, Spec, Src0,
                                    _spill_c3_to_src1, lower, relu, sq)
    from concourse.dve_uop import DveOpSpec

    for op in dve_ops.OPS:
        if op.name == "KANQ_ANT":
            _QOP = op
            return op

    w = Bin(AluOp.ABSOLUTE_DIFF, Src0, C0)
    T = relu(Bin(AluOp.SUBTRACT, C1, w))
    body = _spill_c3_to_src1(sq(T) * (T + C3) * (T + C2))

    def _ref(in0, in1, s0, s1, imm2):
        in0 = np.asarray(in0, np.float32)
        tail = [1] * (in0.ndim - 1)
        c = np.asarray(s0, np.float32).reshape(-1, *tail)
        h = np.asarray(s1, np.float32).reshape(-1, *tail)
        a = np.asarray(in1, np.float32).reshape(-1, *tail)
        T = np.maximum(h - np.abs(in0 - c), 0.0)
        return (T * T * (T + a) * (T + imm2)).astype(np.float32)

    spec = Spec(body=body, reference=_ref)
    row = dve_ops._CUSTOM_DVE_ROW_BASE + len(dve_ops.OPS)
    shas = {}
    for ver in ("v3", "v4"):
        tmp = DveOpSpec(name="KANQ_ANT", opcode=row,
                        uops=lower(spec, ver=ver), rd1_en=True)
        shas[ver] = tmp.sha(ver)
    op = dve_ops.DveOp("KANQ_ANT", spec, subdim=False, uops_sha=shas)
    dve_ops.OPS.append(op)
    dve_ops._SUB_OPCODE_FOR_NAME[op.name] = row
    dve_ops.CUSTOM_DVE_SPECS[op.name] = spec
    _QOP = op
    return op


# --------------------------- host-side math --------------------------------

def _m4(v):
    u = np.abs(v - 2.0)
    r2 = np.maximum(2.0 - u, 0.0)
    r1 = np.maximum(1.0 - u, 0.0)
    return (r2**3 - 4.0 * r1**3) / 6.0


def _silu(x):
    return x / (1.0 + np.exp(-x))


def _psi(s):
    """s: (N,) -> (N, 8) quartic bump features."""
    T = np.maximum(H_ROWS - np.abs(s[:, None] - C_ROWS), 0.0)
    return T * T * (T + A_ROWS) * (T + B_GLOB)


def _fit_layer(coef, ss, s_samp):
    """Per-channel density-weighted lstsq: features -> spline weights.

    coef: (din, dout, 8); ss: (din, dout); s_samp: (n, din)
    returns Wfeat (din, 8, dout) float64
    """
    din, dout, _ = coef.shape
    sgrid = np.linspace(-3.0, 14.0, 1201)
    Psi = _psi(sgrid)                                   # (S, 8)
    Mtgt = np.stack([_m4(sgrid - g) for g in range(8)], 1)
    Wout = np.zeros((din, K, dout))
    for i in range(din):
        hist, edges = np.histogram(s_samp[:, i], bins=120,
                                   range=(-3.0, 14.0), density=True)
        centers = 0.5 * (edges[:-1] + edges[1:])
        wt = np.interp(sgrid, centers, hist) + 1e-3
        sw = np.sqrt(wt)[:, None]
        C, *_ = np.linalg.lstsq(sw * Psi, sw * Mtgt, rcond=None)  # (8, 8)
        Wout[i] = C @ (coef[i] * ss[i][:, None]).T                # (8, dout)
    return Wout


def _host_weights(inputs):
    """Mini-forward for s-samples + per-layer fits; assemble device arrays."""
    x = np.asarray(inputs["x"], np.float64)
    hs = np.transpose(x, (0, 2, 3, 1)).reshape(-1, 12)
    rng = np.random.default_rng(0)
    samp = hs[rng.choice(hs.shape[0], 16384, replace=False)]

    Ws = []
    h = samp
    for li in range(3):
        coef = np.asarray(inputs[f"coef{li}"], np.float64)
        sb = np.asarray(inputs[f"sb{li}"], np.float64)
        ss = np.asarray(inputs[f"ss{li}"], np.float64)
        s = S_SCALE * h + S_BIAS
        Ws.append(_fit_layer(coef, ss, s))
        # exact forward for next layer's sample distribution
        Bsp = np.stack([_m4(s - g) for g in range(8)], -1)       # (n, din, 8)
        h = _silu(h) @ sb + np.einsum('nig,iog->no', Bsp, coef * ss[:, :, None])

    bf = ml_dtypes.bfloat16
    sb0 = np.asarray(inputs["sb0"], np.float64)
    sb1 = np.asarray(inputs["sb1"], np.float64)
    sb2 = np.asarray(inputs["sb2"], np.float64)

    # L0 stationary [108, 128]: rows 0..95 = (k = p//12, i = p%12) features,
    # rows 96..107 = silu base; cols duplicated (o, o+64).
    w0 = np.zeros((108, 128))
    for p in range(96):
        k, i = p // 12, p % 12
        w0[p, 0:64] = Ws[0][i, k]
        w0[p, 64:128] = Ws[0][i, k]
    w0[96:108, 0:64] = sb0
    w0[96:108, 64:128] = sb0

    # mid stationary per page j: [128, mcols]; row p: ch=p%64, k=j+4*(p//64)
    def midw(W, dout, dup):
        mc = 128 if dup else dout
        out = np.zeros((4, 128, mc))
        for j in range(4):
            for grp in range(2):
                k = j + 4 * grp
                blk = W[:, k, :]                       # (64, dout)
                out[j, 64*grp:64*grp+64, 0:dout] = blk
                if dup:
                    out[j, 64*grp:64*grp+64, 64:128] = blk
        return out

    w1 = midw(Ws[1], 64, True)
    w2 = midw(Ws[2], 24, False)
    b1 = np.zeros((64, 128)); b1[:, 0:64] = sb1; b1[:, 64:128] = sb1
    b2 = sb2

    # DVE per-partition scalars
    c0v = np.zeros((96, 1), np.float32); h0v = np.zeros((96, 1), np.float32)
    a0v = np.zeros((96, 1), np.float32)
    for p in range(96):
        k = p // 12
        c0v[p], h0v[p], a0v[p] = C_ROWS[k], H_ROWS[k], A_ROWS[k]
    cv = np.zeros((4, 128, 1), np.float32); hv = np.zeros((4, 128, 1), np.float32)
    av = np.zeros((4, 128, 1), np.float32)
    for j in range(4):
        for p in range(128):
            k = j + 4 * (p // 64)
            cv[j, p], hv[j, p], av[j, p] = C_ROWS[k], H_ROWS[k], A_ROWS[k]

    return {
        "w0": w0.astype(bf), "w1": w1.astype(bf), "b1": b1.astype(bf),
        "w2": w2.astype(bf), "b2": b2.astype(bf),
        "c0v": c0v, "h0v": h0v, "a0v": a0v,
        "cv": cv, "hv": hv, "av": av,
    }


# --------------------------- device program --------------------------------

def _build():
    qop = _register_quartic_op()
    nc = bacc.Bacc("TRN2", target_bir_lowering=False, debug=False,
                   enable_asserts=False, num_devices=N_CORES)

    x_d = nc.dram_tensor("x_in", [B_PER_CORE, 12, HW], FP, kind="ExternalInput").ap()
    out_d = nc.dram_tensor("y_out", [B_PER_CORE, 24, HW], FP, kind="ExternalOutput").ap()
    w0_d = nc.dram_tensor("w0", [108, 128], BF, kind="ExternalInput").ap()
    w1_d = nc.dram_tensor("w1", [4, 128, 128], BF, kind="ExternalInput").ap()
    b1_d = nc.dram_tensor("b1", [64, 128], BF, kind="ExternalInput").ap()
    w2_d = nc.dram_tensor("w2", [4, 128, 24], BF, kind="ExternalInput").ap()
    b2_d = nc.dram_tensor("b2", [64, 24], BF, kind="ExternalInput").ap()
    c0_d = nc.dram_tensor("c0v", [96, 1], FP, kind="ExternalInput").ap()
    h0_d = nc.dram_tensor("h0v", [96, 1], FP, kind="ExternalInput").ap()
    a0_d = nc.dram_tensor("a0v", [96, 1], FP, kind="ExternalInput").ap()
    cv_d = nc.dram_tensor("cv", [4, 128, 1], FP, kind="ExternalInput").ap()
    hv_d = nc.dram_tensor("hv", [4, 128, 1], FP, kind="ExternalInput").ap()
    av_d = nc.dram_tensor("av", [4, 128, 1], FP, kind="ExternalInput").ap()

    with tile.TileContext(nc) as tc:
        with (
            tc.tile_pool(name="consts", bufs=1) as cp,
            tc.tile_pool(name="xr", bufs=2) as xp,
            tc.tile_pool(name="f0", bufs=2) as f0p,
            tc.tile_pool(name="hb", bufs=2) as hp,
            tc.tile_pool(name="ps", bufs=2, space="PSUM") as pp,
        ):
            w0 = cp.tile([108, 128], BF, tag="w0")
            nc.sync.dma_start(w0[:], w0_d[:])
            w1 = [cp.tile([128, 128], BF, tag=f"w1_{j}", name=f"w1_{j}") for j in range(4)]
            w2 = [cp.tile([128, 24], BF, tag=f"w2_{j}", name=f"w2_{j}") for j in range(4)]
            for j in range(4):
                nc.sync.dma_start(w1[j][:], w1_d[j])
                nc.sync.dma_start(w2[j][:], w2_d[j])
            b1 = cp.tile([64, 128], BF, tag="b1")
            nc.sync.dma_start(b1[:], b1_d[:])
            b2 = cp.tile([64, 24], BF, tag="b2")
            nc.sync.dma_start(b2[:], b2_d[:])
            c0v = cp.tile([96, 1], FP, tag="c0v"); nc.sync.dma_start(c0v[:], c0_d[:])
            h0v = cp.tile([96, 1], FP, tag="h0v"); nc.sync.dma_start(h0v[:], h0_d[:])
            a0v = cp.tile([96, 1], FP, tag="a0v"); nc.sync.dma_start(a0v[:], a0_d[:])
            cv = [cp.tile([128, 1], FP, tag=f"cv{j}", name=f"cv{j}") for j in range(4)]
            hv = [cp.tile([128, 1], FP, tag=f"hv{j}", name=f"hv{j}") for j in range(4)]
            av = [cp.tile([128, 1], FP, tag=f"av{j}", name=f"av{j}") for j in range(4)]
            for j in range(4):
                nc.sync.dma_start(cv[j][:], cv_d[j])
                nc.sync.dma_start(hv[j][:], hv_d[j])
                nc.sync.dma_start(av[j][:], av_d[j])
            bias55 = cp.tile([128, 1], FP, tag="bias55")
            nc.gpsimd.memset(bias55[:], S_BIAS)

            def batch_head(b):
                """Load + replicate x, compute s0r and f0 features (per batch)."""
                xr = xp.tile([96, HW], FP, tag="xr")
                for r in range(8):
                    nc.sync.dma_start(xr[12*r:12*(r+1), :], x_d[b])
                s0r = xp.tile([96, HW], FP, tag="s0r")
                nc.scalar.activation(s0r[:], xr[:], AFT.Identity,
                                     bias=bias55[0:96, :], scale=S_SCALE)
                f0t = f0p.tile([108, HW], BF, tag="f0")
                nc.vector._custom_dve(qop, out=f0t[0:96, :], in0=s0r[:],
                                      in1=a0v[:], s0=c0v[:], s1=h0v[:],
                                      imm2=B_GLOB)
                nc.scalar.activation(f0t[96:108, :], xr[0:12, :], AFT.Silu)
                return f0t

            def stage_A(f0t, hb):
                """L0 matmuls + s1/sil1 for one half-batch block."""
                s1 = hp.tile([128, NHB], FP, tag="s1")
                sil1 = hp.tile([64, NHB], BF, tag="sil1")
                for t in range(4):
                    bcols = bass.ts(4*hb + t, NT)
                    lcols = bass.ts(t, NT)
                    ps1 = pp.tile([128, NT], FP, tag="ps1")
                    nc.tensor.matmul(ps1[:], w0[:], f0t[:, bcols],
                                     start=True, stop=True)
                    nc.scalar.activation(s1[:, lcols], ps1[:], AFT.Identity,
                                         bias=bias55[:], scale=S_SCALE)
                    nc.scalar.activation(sil1[:, lcols], ps1[0:64, :], AFT.Silu)
                return s1, sil1

            def feats(s_t, tag):
                f = [hp.tile([128, NHB], BF, tag=f"{tag}_{j}", name=f"{tag}_{j}")
                     for j in range(4)]
                for j in range(4):
                    nc.vector._custom_dve(qop, out=f[j][:], in0=s_t[:],
                                          in1=av[j][:], s0=cv[j][:],
                                          s1=hv[j][:], imm2=B_GLOB)
                return f

            def stage_C(f1, sil1):
                """L1 matmuls + s2/sil2."""
                s2 = hp.tile([128, NHB], FP, tag="s2")
                sil2 = hp.tile([64, NHB], BF, tag="sil2")
                for t in range(4):
                    lcols = bass.ts(t, NT)
                    ps2 = pp.tile([128, NT], FP, tag="ps2")
                    for j in range(4):
                        nc.tensor.matmul(ps2[:], w1[j][:], f1[j][:, lcols],
                                         start=(j == 0), stop=False)
                    nc.tensor.matmul(ps2[:], b1[:], sil1[:, lcols],
                                     start=False, stop=True)
                    nc.scalar.activation(s2[:, lcols], ps2[:], AFT.Identity,
                                         bias=bias55[:], scale=S_SCALE)
                    nc.scalar.activation(sil2[:, lcols], ps2[0:64, :], AFT.Silu)
                return s2, sil2

            def stage_E(b, hb, f2, sil2):
                """L2 matmuls + output DMA."""
                for t in range(4):
                    bcols = bass.ts(4*hb + t, NT)
                    lcols = bass.ts(t, NT)
                    ps3 = pp.tile([24, NT], FP, tag="ps3")
                    for j in range(4):
                        nc.tensor.matmul(ps3[:], w2[j][:], f2[j][:, lcols],
                                         start=(j == 0), stop=False)
                    nc.tensor.matmul(ps3[:], b2[:], sil2[:, lcols],
                                     start=False, stop=True)
                    yt = hp.tile([24, NT], FP, tag="yt")
                    nc.scalar.activation(yt[:], ps3[:], AFT.Identity)
                    nc.sync.dma_start(out_d[b, :, bcols], yt[:])

            # software pipeline: stage A runs one half-batch ahead
            blocks = [(b, hb) for b in range(B_PER_CORE) for hb in range(2)]
            f0t = batch_head(0)
            pend_A = stage_A(f0t, 0)
            for idx, (b, hb) in enumerate(blocks):
                s1, sil1 = pend_A
                f1 = feats(s1, "f1")
                s2, sil2 = stage_C(f1, sil1)
                f2 = feats(s2, "f2")
                if idx + 1 < len(blocks):
                    nb, nhb = blocks[idx + 1]
                    if nhb == 0:
                        f0t = batch_head(nb)
                    pend_A = stage_A(f0t, nhb)
                stage_E(b, hb, f2, sil2)

    nc.compile()
    return nc


# ------------------------------ entry points -------------------------------

def kernel(x, grid0, coef0, sb0, ss0, grid1, coef1, sb1, ss1, grid2, coef2, sb2, ss2):
    if "nc" not in _CACHE:
        _CACHE["nc"] = _build()
    nc = _CACHE["nc"]

    inputs = {"x": x, "coef0": coef0, "sb0": sb0, "ss0": ss0,
              "coef1": coef1, "sb1": sb1, "ss1": ss1,
              "coef2": coef2, "sb2": sb2, "ss2": ss2}
    consts = _host_weights(inputs)

    xf = np.asarray(x, np.float32).reshape(32, 12, HW)
    maps = []
    for c in range(N_CORES):
        m = dict(consts)
        m["x_in"] = np.ascontiguousarray(xf[c*B_PER_CORE:(c+1)*B_PER_CORE])
        maps.append(m)
    res = run_bass_kernel_spmd(nc, maps, core_ids=list(range(N_CORES)))
    _CACHE["maps"] = maps
    out = np.empty((32, 24, HW), np.float32)
    for c in range(N_CORES):
        out[c*B_PER_CORE:(c+1)*B_PER_CORE] = res.results[c]["y_out"]
    return out.reshape(32, 24, 64, 64)


def _install_ntff_hook():
    import sys, types
    if "antenv.axon_hooks" in sys.modules:
        return
    state = {"hook": None}
    mod = types.ModuleType("antenv.axon_hooks")
    mod.set_axon_ntff_profile_hook = lambda h: state.__setitem__("hook", h)
    mod.get_axon_ntff_profile_hook = lambda: state["hook"]
    sys.modules["antenv.axon_hooks"] = mod
    import antenv
    antenv.axon_hooks = mod
    from trn_agent_boot.trn_boot import _ntff_profile_via_ctypes
    hook = _ntff_profile_via_ctypes("/opt/axon/libaxon_pjrt.so")
    if hook is not None:
        mod.set_axon_ntff_profile_hook(hook)


def profile():
    _install_ntff_hook()
    nc = _CACHE["nc"]
    res = run_bass_kernel_spmd(nc, _CACHE["maps"], core_ids=list(range(N_CORES)),
                               trace=True)
    return res.exec_time_ns, getattr(res, "instructions_and_trace", None)


# revision 12
# speedup vs baseline: 1.3652x; 1.2359x over previous
"""Trainium2 Bass kernel for nn_KANCouplingNet (3-layer KAN MLP, 12-64-64-24).

Each KAN layer: y = silu(h) @ sb + B(s) @ W with s = h/0.4 + 5.5 and B the
cubic B-spline basis (8 functions per channel).  The exact cardinal basis
needs >=2 DVE ops per basis value (the M4 two-cube formula exceeds the 8-op
DVE pipeline), so instead the basis is replaced by 8 quartic bump features
per channel:

    psi_k(s) = T^2 (T + a_k)(T + b),   T = relu(h_k - |s - c_k|)

computed by ONE custom DVE instruction (8 ALU ops, per-partition c/h/a via
scalar slots, b via imm2).  The transform features->basis is re-fit on the
host per channel with empirical s-density weighting (mini-forward on a pixel
subsample); end-to-end validation vs the exact reference gives ~4.5e-4
relative error (gate 2e-2), robust to bf16 features/weights (5.3e-4).

This halves DVE work vs the exact two-cube kernel (1 op/basis value) and
nearly halves the matmul count.  Sharding: pure data parallel over batch
(32 images -> 4 per core); features built per half-batch (2048 px) blocks,
software-pipelined one stage ahead so TensorE never stalls on the DVE.
"""
import numpy as np
import ml_dtypes

import concourse.bacc as bacc
import concourse.bass as bass
import concourse.mybir as mybir
import concourse.tile as tile
from concourse.bass_utils import run_bass_kernel_spmd

FP = mybir.dt.float32
BF = mybir.dt.bfloat16
AFT = mybir.ActivationFunctionType

N_CORES = 8
B_PER_CORE = 4
HW = 64 * 64            # 4096 px per image
NT = 512                # matmul moving dim (one PSUM bank)
NHB = 2048              # feature-block pixels (half batch)
S_SCALE = 2.5
S_BIAS = 5.5
K = 8                   # feature rows per input channel

# Generator params (offline fit; end-to-end 4.5e-4 vs exact reference).
A_ROWS = np.array([-0.3615, -0.4926, -0.5196, -0.5232,
                   -0.5232, -0.5196, -0.4926, -0.3615], np.float64)
B_GLOB = -2.5643
C_ROWS = np.arange(8, dtype=np.float64) + 2.0
H_ROWS = np.full(8, 2.0, np.float64)

# psi is scale-invariant: T(s) = S_SCALE * T'(h) with T' built from raw h via
# c' = (c - S_BIAS)/S_SCALE etc., and psi = S_SCALE^4 * psi'.  The device op
# therefore consumes raw x / raw pre-activations (no s staging act), with the
# S_SCALE^4 factor folded into the matmul weights.
C_DEV = (C_ROWS - S_BIAS) / S_SCALE
H_DEV = H_ROWS / S_SCALE
A_DEV = A_ROWS / S_SCALE
B_DEV = B_GLOB / S_SCALE
W_FOLD = S_SCALE ** 4

_CACHE = {}
_QOP = None


def _register_quartic_op():
    """out = T^2 (T + in1)(T + imm2), T = relu(s1 - |in0 - s0|)."""
    global _QOP
    if _QOP is not None:
        return _QOP
    from concourse import dve_ops
    from concourse.dve_spec import (AluOp, Bin, C0, C1, C2, C3, Spec, Src0,
                                    _spill_c3_to_src1, lower, relu, sq)
    from concourse.dve_uop import DveOpSpec

    for op in dve_ops.OPS:
        if op.name == "KANQ_ANT":
            _QOP = op
            return op

    w = Bin(AluOp.ABSOLUTE_DIFF, Src0, C0)
    T = relu(Bin(AluOp.SUBTRACT, C1, w))
    body = _spill_c3_to_src1(sq(T) * (T + C3) * (T + C2))

    def _ref(in0, in1, s0, s1, imm2):
        in0 = np.asarray(in0, np.float32)
        tail = [1] * (in0.ndim - 1)
        c = np.asarray(s0, np.float32).reshape(-1, *tail)
        h = np.asarray(s1, np.float32).reshape(-1, *tail)
        a = np.asarray(in1, np.float32).reshape(-1, *tail)
        T = np.maximum(h - np.abs(in0 - c), 0.0)
        return (T * T * (T + a) * (T + imm2)).astype(np.float32)

    spec = Spec(body=body, reference=_ref)
    row = dve_ops._CUSTOM_DVE_ROW_BASE + len(dve_ops.OPS)
    shas = {}
    for ver in ("v3", "v4"):
        tmp = DveOpSpec(name="KANQ_ANT", opcode=row,
                        uops=lower(spec, ver=ver), rd1_en=True)
        shas[ver] = tmp.sha(ver)
    op = dve_ops.DveOp("KANQ_ANT", spec, subdim=False, uops_sha=shas)
    dve_ops.OPS.append(op)
    dve_ops._SUB_OPCODE_FOR_NAME[op.name] = row
    dve_ops.CUSTOM_DVE_SPECS[op.name] = spec
    _QOP = op
    return op


# --------------------------- host-side math --------------------------------

def _m4(v):
    u = np.abs(v - 2.0)
    r2 = np.maximum(2.0 - u, 0.0)
    r1 = np.maximum(1.0 - u, 0.0)
    return (r2**3 - 4.0 * r1**3) / 6.0


def _silu(x):
    return x / (1.0 + np.exp(-x))


def _psi(s):
    """s: (N,) -> (N, 8) quartic bump features."""
    T = np.maximum(H_ROWS - np.abs(s[:, None] - C_ROWS), 0.0)
    return T * T * (T + A_ROWS) * (T + B_GLOB)


def _fit_layer(coef, ss, s_samp):
    """Per-channel density-weighted lstsq: features -> spline weights.

    coef: (din, dout, 8); ss: (din, dout); s_samp: (n, din)
    returns Wfeat (din, 8, dout) float64
    """
    din, dout, _ = coef.shape
    sgrid = np.linspace(-3.0, 14.0, 1201)
    Psi = _psi(sgrid)                                   # (S, 8)
    Mtgt = np.stack([_m4(sgrid - g) for g in range(8)], 1)
    Wout = np.zeros((din, K, dout))
    for i in range(din):
        hist, edges = np.histogram(s_samp[:, i], bins=120,
                                   range=(-3.0, 14.0), density=True)
        centers = 0.5 * (edges[:-1] + edges[1:])
        wt = np.interp(sgrid, centers, hist) + 1e-3
        sw = np.sqrt(wt)[:, None]
        C, *_ = np.linalg.lstsq(sw * Psi, sw * Mtgt, rcond=None)  # (8, 8)
        Wout[i] = C @ (coef[i] * ss[i][:, None]).T                # (8, dout)
    return Wout


def _host_weights(inputs):
    """Mini-forward for s-samples + per-layer fits; assemble device arrays."""
    x = np.asarray(inputs["x"], np.float64)
    hs = np.transpose(x, (0, 2, 3, 1)).reshape(-1, 12)
    rng = np.random.default_rng(0)
    samp = hs[rng.choice(hs.shape[0], 16384, replace=False)]

    Ws = []
    h = samp
    for li in range(3):
        coef = np.asarray(inputs[f"coef{li}"], np.float64)
        sb = np.asarray(inputs[f"sb{li}"], np.float64)
        ss = np.asarray(inputs[f"ss{li}"], np.float64)
        s = S_SCALE * h + S_BIAS
        Ws.append(_fit_layer(coef, ss, s))
        # exact forward for next layer's sample distribution
        Bsp = np.stack([_m4(s - g) for g in range(8)], -1)       # (n, din, 8)
        h = _silu(h) @ sb + np.einsum('nig,iog->no', Bsp, coef * ss[:, :, None])

    bf = ml_dtypes.bfloat16
    sb0 = np.asarray(inputs["sb0"], np.float64)
    sb1 = np.asarray(inputs["sb1"], np.float64)
    sb2 = np.asarray(inputs["sb2"], np.float64)

    # L0 stationary [108, 128]: rows 0..95 = (k = p//12, i = p%12) features,
    # rows 96..107 = silu base; cols duplicated (o, o+64).
    w0 = np.zeros((108, 128))
    for p in range(96):
        k, i = p // 12, p % 12
        w0[p, 0:64] = W_FOLD * Ws[0][i, k]
        w0[p, 64:128] = W_FOLD * Ws[0][i, k]
    w0[96:108, 0:64] = sb0
    w0[96:108, 64:128] = sb0

    # mid stationary per page j: [128, mcols]; row p: ch=p%64, k=j+4*(p//64)
    def midw(W, dout, dup):
        mc = 128 if dup else dout
        out = np.zeros((4, 128, mc))
        for j in range(4):
            for grp in range(2):
                k = j + 4 * grp
                blk = W_FOLD * W[:, k, :]              # (64, dout)
                out[j, 64*grp:64*grp+64, 0:dout] = blk
                if dup:
                    out[j, 64*grp:64*grp+64, 64:128] = blk
        return out

    w1 = midw(Ws[1], 64, True)
    w2 = midw(Ws[2], 24, False)
    b1 = np.zeros((64, 128)); b1[:, 0:64] = sb1; b1[:, 64:128] = sb1
    b2 = sb2

    # DVE per-partition scalars (raw-input scale)
    c0v = np.zeros((96, 1), np.float32); h0v = np.zeros((96, 1), np.float32)
    a0v = np.zeros((96, 1), np.float32)
    for p in range(96):
        k = p // 12
        c0v[p], h0v[p], a0v[p] = C_DEV[k], H_DEV[k], A_DEV[k]
    cv = np.zeros((4, 128, 1), np.float32); hv = np.zeros((4, 128, 1), np.float32)
    av = np.zeros((4, 128, 1), np.float32)
    for j in range(4):
        for p in range(128):
            k = j + 4 * (p // 64)
            cv[j, p], hv[j, p], av[j, p] = C_DEV[k], H_DEV[k], A_DEV[k]

    return {
        "w0": w0.astype(bf), "w1": w1.astype(bf), "b1": b1.astype(bf),
        "w2": w2.astype(bf), "b2": b2.astype(bf),
        "c0v": c0v, "h0v": h0v, "a0v": a0v,
        "cv": cv, "hv": hv, "av": av,
    }


# --------------------------- device program --------------------------------

def _build():
    qop = _register_quartic_op()
    nc = bacc.Bacc("TRN2", target_bir_lowering=False, debug=False,
                   enable_asserts=False, num_devices=N_CORES)

    x_d = nc.dram_tensor("x_in", [B_PER_CORE, 12, HW], FP, kind="ExternalInput").ap()
    out_d = nc.dram_tensor("y_out", [B_PER_CORE, 24, HW], FP, kind="ExternalOutput").ap()
    w0_d = nc.dram_tensor("w0", [108, 128], BF, kind="ExternalInput").ap()
    w1_d = nc.dram_tensor("w1", [4, 128, 128], BF, kind="ExternalInput").ap()
    b1_d = nc.dram_tensor("b1", [64, 128], BF, kind="ExternalInput").ap()
    w2_d = nc.dram_tensor("w2", [4, 128, 24], BF, kind="ExternalInput").ap()
    b2_d = nc.dram_tensor("b2", [64, 24], BF, kind="ExternalInput").ap()
    c0_d = nc.dram_tensor("c0v", [96, 1], FP, kind="ExternalInput").ap()
    h0_d = nc.dram_tensor("h0v", [96, 1], FP, kind="ExternalInput").ap()
    a0_d = nc.dram_tensor("a0v", [96, 1], FP, kind="ExternalInput").ap()
    cv_d = nc.dram_tensor("cv", [4, 128, 1], FP, kind="ExternalInput").ap()
    hv_d = nc.dram_tensor("hv", [4, 128, 1], FP, kind="ExternalInput").ap()
    av_d = nc.dram_tensor("av", [4, 128, 1], FP, kind="ExternalInput").ap()

    with tile.TileContext(nc) as tc:
        with (
            tc.tile_pool(name="consts", bufs=1) as cp,
            tc.tile_pool(name="xr", bufs=2) as xp,
            tc.tile_pool(name="f0", bufs=2) as f0p,
            tc.tile_pool(name="hb", bufs=2) as hp,
            tc.tile_pool(name="ps", bufs=2, space="PSUM") as pp,
        ):
            w0 = cp.tile([108, 128], BF, tag="w0")
            nc.sync.dma_start(w0[:], w0_d[:])
            w1 = [cp.tile([128, 128], BF, tag=f"w1_{j}", name=f"w1_{j}") for j in range(4)]
            w2 = [cp.tile([128, 24], BF, tag=f"w2_{j}", name=f"w2_{j}") for j in range(4)]
            for j in range(4):
                nc.sync.dma_start(w1[j][:], w1_d[j])
                nc.sync.dma_start(w2[j][:], w2_d[j])
            b1 = cp.tile([64, 128], BF, tag="b1")
            nc.sync.dma_start(b1[:], b1_d[:])
            b2 = cp.tile([64, 24], BF, tag="b2")
            nc.sync.dma_start(b2[:], b2_d[:])
            c0v = cp.tile([96, 1], FP, tag="c0v"); nc.sync.dma_start(c0v[:], c0_d[:])
            h0v = cp.tile([96, 1], FP, tag="h0v"); nc.sync.dma_start(h0v[:], h0_d[:])
            a0v = cp.tile([96, 1], FP, tag="a0v"); nc.sync.dma_start(a0v[:], a0_d[:])
            cv = [cp.tile([128, 1], FP, tag=f"cv{j}", name=f"cv{j}") for j in range(4)]
            hv = [cp.tile([128, 1], FP, tag=f"hv{j}", name=f"hv{j}") for j in range(4)]
            av = [cp.tile([128, 1], FP, tag=f"av{j}", name=f"av{j}") for j in range(4)]
            for j in range(4):
                nc.sync.dma_start(cv[j][:], cv_d[j])
                nc.sync.dma_start(hv[j][:], hv_d[j])
                nc.sync.dma_start(av[j][:], av_d[j])
            def batch_head(b):
                """Load + replicate raw x, f0 features + silu base (per batch)."""
                xr = xp.tile([96, HW], FP, tag="xr")
                for r in range(8):
                    nc.sync.dma_start(xr[12*r:12*(r+1), :], x_d[b])
                f0t = f0p.tile([108, HW], BF, tag="f0")
                nc.vector._custom_dve(qop, out=f0t[0:96, :], in0=xr[:],
                                      in1=a0v[:], s0=c0v[:], s1=h0v[:],
                                      imm2=B_DEV)
                nc.scalar.activation(f0t[96:108, :], xr[0:12, :], AFT.Silu)
                return f0t

            def stage_A(f0t, hb):
                """L0 matmuls; evacuate h1 (gpsimd) + silu (scalar)."""
                s1 = hp.tile([128, NHB], FP, tag="s1")
                sil1 = hp.tile([64, NHB], BF, tag="sil1")
                for t in range(4):
                    bcols = bass.ts(4*hb + t, NT)
                    lcols = bass.ts(t, NT)
                    ps1 = pp.tile([128, NT], FP, tag="ps1")
                    nc.tensor.matmul(ps1[:], w0[:], f0t[:, bcols],
                                     start=True, stop=True)
                    nc.scalar.activation(s1[:, lcols], ps1[:], AFT.Identity)
                    nc.scalar.activation(sil1[:, lcols], ps1[0:64, :], AFT.Silu)
                return s1, sil1

            def feats(s_t, tag):
                f = [hp.tile([128, NHB], BF, tag=f"{tag}_{j}", name=f"{tag}_{j}")
                     for j in range(4)]
                for j in range(4):
                    nc.vector._custom_dve(qop, out=f[j][:], in0=s_t[:],
                                          in1=av[j][:], s0=cv[j][:],
                                          s1=hv[j][:], imm2=B_DEV)
                return f

            def stage_C(f1, sil1):
                """L1 matmuls; evacuate h2 + silu."""
                s2 = hp.tile([128, NHB], FP, tag="s2")
                sil2 = hp.tile([64, NHB], BF, tag="sil2")
                for t in range(4):
                    lcols = bass.ts(t, NT)
                    ps2 = pp.tile([128, NT], FP, tag="ps2")
                    for j in range(4):
                        nc.tensor.matmul(ps2[:], w1[j][:], f1[j][:, lcols],
                                         start=(j == 0), stop=False)
                    nc.tensor.matmul(ps2[:], b1[:], sil1[:, lcols],
                                     start=False, stop=True)
                    nc.scalar.activation(s2[:, lcols], ps2[:], AFT.Identity)
                    nc.scalar.activation(sil2[:, lcols], ps2[0:64, :], AFT.Silu)
                return s2, sil2

            def stage_E(b, hb, f2, sil2):
                """L2 matmuls + output staging DMA."""
                for t in range(4):
                    bcols = bass.ts(4*hb + t, NT)
                    lcols = bass.ts(t, NT)
                    ps3 = pp.tile([24, NT], FP, tag="ps3")
                    for j in range(4):
                        nc.tensor.matmul(ps3[:], w2[j][:], f2[j][:, lcols],
                                         start=(j == 0), stop=False)
                    nc.tensor.matmul(ps3[:], b2[:], sil2[:, lcols],
                                     start=False, stop=True)
                    yt = hp.tile([24, NT], FP, tag="yt")
                    nc.scalar.activation(yt[:], ps3[:], AFT.Identity)
                    nc.sync.dma_start(out_d[b, :, bcols], yt[:])

            # software pipeline: emit next block's L0 (stage A) right after
            # this block's f1 so TensorE always has ready work queued ahead
            # of the f2-dependent L2 matmuls.
            blocks = [(b, hb) for b in range(B_PER_CORE) for hb in range(2)]
            f0t = batch_head(0)
            pend_A = stage_A(f0t, 0)
            for idx, (b, hb) in enumerate(blocks):
                s1, sil1 = pend_A
                f1 = feats(s1, "f1")
                if idx + 1 < len(blocks):
                    nb, nhb = blocks[idx + 1]
                    if nhb == 0:
                        f0t = batch_head(nb)
                    pend_A = stage_A(f0t, nhb)
                s2, sil2 = stage_C(f1, sil1)
                f2 = feats(s2, "f2")
                stage_E(b, hb, f2, sil2)

    nc.compile()
    return nc


# ------------------------------ entry points -------------------------------

def kernel(x, grid0, coef0, sb0, ss0, grid1, coef1, sb1, ss1, grid2, coef2, sb2, ss2):
    if "nc" not in _CACHE:
        _CACHE["nc"] = _build()
    nc = _CACHE["nc"]

    inputs = {"x": x, "coef0": coef0, "sb0": sb0, "ss0": ss0,
              "coef1": coef1, "sb1": sb1, "ss1": ss1,
              "coef2": coef2, "sb2": sb2, "ss2": ss2}
    consts = _host_weights(inputs)

    xf = np.asarray(x, np.float32).reshape(32, 12, HW)
    maps = []
    for c in range(N_CORES):
        m = dict(consts)
        m["x_in"] = np.ascontiguousarray(xf[c*B_PER_CORE:(c+1)*B_PER_CORE])
        maps.append(m)
    res = run_bass_kernel_spmd(nc, maps, core_ids=list(range(N_CORES)))
    _CACHE["maps"] = maps
    out = np.empty((32, 24, HW), np.float32)
    for c in range(N_CORES):
        out[c*B_PER_CORE:(c+1)*B_PER_CORE] = res.results[c]["y_out"]
    return out.reshape(32, 24, 64, 64)


def _install_ntff_hook():
    import sys, types
    if "antenv.axon_hooks" in sys.modules:
        return
    state = {"hook": None}
    mod = types.ModuleType("antenv.axon_hooks")
    mod.set_axon_ntff_profile_hook = lambda h: state.__setitem__("hook", h)
    mod.get_axon_ntff_profile_hook = lambda: state["hook"]
    sys.modules["antenv.axon_hooks"] = mod
    import antenv
    antenv.axon_hooks = mod
    from trn_agent_boot.trn_boot import _ntff_profile_via_ctypes
    hook = _ntff_profile_via_ctypes("/opt/axon/libaxon_pjrt.so")
    if hook is not None:
        mod.set_axon_ntff_profile_hook(hook)


def profile():
    _install_ntff_hook()
    nc = _CACHE["nc"]
    res = run_bass_kernel_spmd(nc, _CACHE["maps"], core_ids=list(range(N_CORES)),
                               trace=True)
    return res.exec_time_ns, getattr(res, "instructions_and_trace", None)


# revision 15
# speedup vs baseline: 1.6359x; 1.1983x over previous
"""Trainium2 Bass kernel for nn_KANCouplingNet (3-layer KAN MLP, 12-64-64-24).

Each KAN layer: y = silu(h) @ sb + B(s) @ W with s = h/0.4 + 5.5 and B the
cubic B-spline basis (8 functions per channel).  The exact cardinal basis
needs >=2 DVE ops per basis value (the M4 two-cube formula exceeds the 8-op
DVE pipeline), so instead the basis is replaced by 8 quartic bump features
per channel:

    psi_k(s) = T^2 (T + a_k)(T + b),   T = relu(h_k - |s - c_k|)

computed by ONE custom DVE instruction (8 ALU ops, per-partition c/h/a via
scalar slots, b via imm2).  The transform features->basis is re-fit on the
host per channel with empirical s-density weighting (mini-forward on a pixel
subsample); end-to-end validation vs the exact reference gives ~4.5e-4
relative error (gate 2e-2), robust to bf16 features/weights (5.3e-4).

This halves DVE work vs the exact two-cube kernel (1 op/basis value) and
nearly halves the matmul count.  Sharding: pure data parallel over batch
(32 images -> 4 per core); features built per half-batch (2048 px) blocks,
software-pipelined one stage ahead so TensorE never stalls on the DVE.
"""
import numpy as np
import ml_dtypes

import concourse.bacc as bacc
import concourse.bass as bass
import concourse.mybir as mybir
import concourse.tile as tile
from concourse.bass_utils import run_bass_kernel_spmd

FP = mybir.dt.float32
BF = mybir.dt.bfloat16
AFT = mybir.ActivationFunctionType

N_CORES = 8
B_PER_CORE = 4
HW = 64 * 64            # 4096 px per image
NT = 512                # matmul moving dim (one PSUM bank)
NHB = 2048              # feature-block pixels (half batch)
S_SCALE = 2.5
S_BIAS = 5.5
K = 8                   # feature rows per input channel

# Generator params (offline fit; end-to-end 4.5e-4 vs exact reference).
A_ROWS = np.array([-0.3615, -0.4926, -0.5196, -0.5232,
                   -0.5232, -0.5196, -0.4926, -0.3615], np.float64)
B_GLOB = -2.5643
C_ROWS = np.arange(8, dtype=np.float64) + 2.0
H_ROWS = np.full(8, 2.0, np.float64)

# psi is scale-invariant: T(s) = S_SCALE * T'(h) with T' built from raw h via
# c' = (c - S_BIAS)/S_SCALE etc., and psi = S_SCALE^4 * psi'.  The device op
# therefore consumes raw x / raw pre-activations (no s staging act), with the
# S_SCALE^4 factor folded into the matmul weights.
C_DEV = (C_ROWS - S_BIAS) / S_SCALE
H_DEV = H_ROWS / S_SCALE
A_DEV = A_ROWS / S_SCALE
B_DEV = B_GLOB / S_SCALE
W_FOLD = S_SCALE ** 4

_CACHE = {}
_QOP = None


def _register_quartic_op():
    """out = T^2 (T + in1)(T + imm2), T = relu(s1 - |in0 - s0|)."""
    global _QOP
    if _QOP is not None:
        return _QOP
    from concourse import dve_ops
    from concourse.dve_spec import (AluOp, Bin, C0, C1, C2, C3, Spec, Src0,
                                    _spill_c3_to_src1, lower, relu, sq)
    from concourse.dve_uop import DveOpSpec

    for op in dve_ops.OPS:
        if op.name == "KANQ_ANT":
            _QOP = op
            return op

    w = Bin(AluOp.ABSOLUTE_DIFF, Src0, C0)
    T = relu(Bin(AluOp.SUBTRACT, C1, w))
    body = _spill_c3_to_src1(sq(T) * (T + C3) * (T + C2))

    def _ref(in0, in1, s0, s1, imm2):
        in0 = np.asarray(in0, np.float32)
        tail = [1] * (in0.ndim - 1)
        c = np.asarray(s0, np.float32).reshape(-1, *tail)
        h = np.asarray(s1, np.float32).reshape(-1, *tail)
        a = np.asarray(in1, np.float32).reshape(-1, *tail)
        T = np.maximum(h - np.abs(in0 - c), 0.0)
        return (T * T * (T + a) * (T + imm2)).astype(np.float32)

    spec = Spec(body=body, reference=_ref)
    row = dve_ops._CUSTOM_DVE_ROW_BASE + len(dve_ops.OPS)
    shas = {}
    for ver in ("v3", "v4"):
        tmp = DveOpSpec(name="KANQ_ANT", opcode=row,
                        uops=lower(spec, ver=ver), rd1_en=True)
        shas[ver] = tmp.sha(ver)
    op = dve_ops.DveOp("KANQ_ANT", spec, subdim=False, uops_sha=shas)
    dve_ops.OPS.append(op)
    dve_ops._SUB_OPCODE_FOR_NAME[op.name] = row
    dve_ops.CUSTOM_DVE_SPECS[op.name] = spec
    _QOP = op
    return op


# --------------------------- host-side math --------------------------------

def _m4(v):
    u = np.abs(v - 2.0)
    r2 = np.maximum(2.0 - u, 0.0)
    r1 = np.maximum(1.0 - u, 0.0)
    return (r2**3 - 4.0 * r1**3) / 6.0


def _silu(x):
    return x / (1.0 + np.exp(-x))


def _psi(s):
    """s: (N,) -> (N, 8) quartic bump features."""
    T = np.maximum(H_ROWS - np.abs(s[:, None] - C_ROWS), 0.0)
    return T * T * (T + A_ROWS) * (T + B_GLOB)


def _fit_layer(coef, ss, s_samp):
    """Per-channel density-weighted lstsq: features -> spline weights.

    coef: (din, dout, 8); ss: (din, dout); s_samp: (n, din)
    returns Wfeat (din, 8, dout) float64
    """
    din, dout, _ = coef.shape
    sgrid = np.linspace(-3.0, 14.0, 1201)
    Psi = _psi(sgrid)                                   # (S, 8)
    Mtgt = np.stack([_m4(sgrid - g) for g in range(8)], 1)
    Wout = np.zeros((din, K, dout))
    for i in range(din):
        hist, edges = np.histogram(s_samp[:, i], bins=120,
                                   range=(-3.0, 14.0), density=True)
        centers = 0.5 * (edges[:-1] + edges[1:])
        wt = np.interp(sgrid, centers, hist) + 1e-3
        sw = np.sqrt(wt)[:, None]
        C, *_ = np.linalg.lstsq(sw * Psi, sw * Mtgt, rcond=None)  # (8, 8)
        Wout[i] = C @ (coef[i] * ss[i][:, None]).T                # (8, dout)
    return Wout


def _host_weights(inputs):
    """Mini-forward for s-samples + per-layer fits; assemble device arrays."""
    x = np.asarray(inputs["x"], np.float64)
    hs = np.transpose(x, (0, 2, 3, 1)).reshape(-1, 12)
    rng = np.random.default_rng(0)
    samp = hs[rng.choice(hs.shape[0], 16384, replace=False)]

    Ws = []
    h = samp
    for li in range(3):
        coef = np.asarray(inputs[f"coef{li}"], np.float64)
        sb = np.asarray(inputs[f"sb{li}"], np.float64)
        ss = np.asarray(inputs[f"ss{li}"], np.float64)
        s = S_SCALE * h + S_BIAS
        Ws.append(_fit_layer(coef, ss, s))
        # exact forward for next layer's sample distribution
        Bsp = np.stack([_m4(s - g) for g in range(8)], -1)       # (n, din, 8)
        h = _silu(h) @ sb + np.einsum('nig,iog->no', Bsp, coef * ss[:, :, None])

    bf = ml_dtypes.bfloat16
    sb0 = np.asarray(inputs["sb0"], np.float64)
    sb1 = np.asarray(inputs["sb1"], np.float64)
    sb2 = np.asarray(inputs["sb2"], np.float64)

    # L0 stationary [108, 128]: rows 0..95 = (k = p//12, i = p%12) features,
    # rows 96..107 = silu base; cols duplicated (o, o+64).
    w0 = np.zeros((108, 128))
    for p in range(96):
        k, i = p // 12, p % 12
        w0[p, 0:64] = W_FOLD * Ws[0][i, k]
        w0[p, 64:128] = W_FOLD * Ws[0][i, k]
    w0[96:108, 0:64] = sb0
    w0[96:108, 64:128] = sb0

    # mid stationary per page j: [128, mcols]; row p: ch=p%64, k=j+4*(p//64)
    def midw(W, dout, dup):
        mc = 128 if dup else dout
        out = np.zeros((4, 128, mc))
        for j in range(4):
            for grp in range(2):
                k = j + 4 * grp
                blk = W_FOLD * W[:, k, :]              # (64, dout)
                out[j, 64*grp:64*grp+64, 0:dout] = blk
                if dup:
                    out[j, 64*grp:64*grp+64, 64:128] = blk
        return out

    w1 = midw(Ws[1], 64, True)
    w2 = midw(Ws[2], 24, False)
    b1 = np.zeros((64, 128)); b1[:, 0:64] = sb1; b1[:, 64:128] = sb1
    b2 = sb2

    # DVE per-partition scalars (raw-input scale)
    c0v = np.zeros((96, 1), np.float32); h0v = np.zeros((96, 1), np.float32)
    a0v = np.zeros((96, 1), np.float32)
    for p in range(96):
        k = p // 12
        c0v[p], h0v[p], a0v[p] = C_DEV[k], H_DEV[k], A_DEV[k]
    cv = np.zeros((4, 128, 1), np.float32); hv = np.zeros((4, 128, 1), np.float32)
    av = np.zeros((4, 128, 1), np.float32)
    for j in range(4):
        for p in range(128):
            k = j + 4 * (p // 64)
            cv[j, p], hv[j, p], av[j, p] = C_DEV[k], H_DEV[k], A_DEV[k]

    return {
        "w0": w0.astype(bf), "w1": w1.astype(bf), "b1": b1.astype(bf),
        "w2": w2.astype(bf), "b2": b2.astype(bf),
        "c0v": c0v, "h0v": h0v, "a0v": a0v,
        "cv": cv, "hv": hv, "av": av,
    }


# --------------------------- device program --------------------------------

def _build():
    qop = _register_quartic_op()
    nc = bacc.Bacc("TRN2", target_bir_lowering=False, debug=False,
                   enable_asserts=False, num_devices=N_CORES)

    x_d = nc.dram_tensor("x_in", [B_PER_CORE, 12, HW], FP, kind="ExternalInput").ap()
    out_d = nc.dram_tensor("y_out", [B_PER_CORE, 24, HW], FP, kind="ExternalOutput").ap()
    w0_d = nc.dram_tensor("w0", [108, 128], BF, kind="ExternalInput").ap()
    w1_d = nc.dram_tensor("w1", [4, 128, 128], BF, kind="ExternalInput").ap()
    b1_d = nc.dram_tensor("b1", [64, 128], BF, kind="ExternalInput").ap()
    w2_d = nc.dram_tensor("w2", [4, 128, 24], BF, kind="ExternalInput").ap()
    b2_d = nc.dram_tensor("b2", [64, 24], BF, kind="ExternalInput").ap()
    c0_d = nc.dram_tensor("c0v", [96, 1], FP, kind="ExternalInput").ap()
    h0_d = nc.dram_tensor("h0v", [96, 1], FP, kind="ExternalInput").ap()
    a0_d = nc.dram_tensor("a0v", [96, 1], FP, kind="ExternalInput").ap()
    cv_d = nc.dram_tensor("cv", [4, 128, 1], FP, kind="ExternalInput").ap()
    hv_d = nc.dram_tensor("hv", [4, 128, 1], FP, kind="ExternalInput").ap()
    av_d = nc.dram_tensor("av", [4, 128, 1], FP, kind="ExternalInput").ap()

    with tile.TileContext(nc) as tc:
        with (
            tc.tile_pool(name="consts", bufs=1) as cp,
            tc.tile_pool(name="xr", bufs=2) as xp,
            tc.tile_pool(name="f0", bufs=2) as f0p,
            tc.tile_pool(name="hb", bufs=2) as hp,
            tc.tile_pool(name="ps", bufs=2, space="PSUM") as pp,
        ):
            w0 = cp.tile([108, 128], BF, tag="w0")
            nc.sync.dma_start(w0[:], w0_d[:])
            w1 = [cp.tile([128, 128], BF, tag=f"w1_{j}", name=f"w1_{j}") for j in range(4)]
            w2 = [cp.tile([128, 24], BF, tag=f"w2_{j}", name=f"w2_{j}") for j in range(4)]
            for j in range(4):
                nc.sync.dma_start(w1[j][:], w1_d[j])
                nc.sync.dma_start(w2[j][:], w2_d[j])
            b1 = cp.tile([64, 128], BF, tag="b1")
            nc.sync.dma_start(b1[:], b1_d[:])
            b2 = cp.tile([64, 24], BF, tag="b2")
            nc.sync.dma_start(b2[:], b2_d[:])
            c0v = cp.tile([96, 1], FP, tag="c0v"); nc.sync.dma_start(c0v[:], c0_d[:])
            h0v = cp.tile([96, 1], FP, tag="h0v"); nc.sync.dma_start(h0v[:], h0_d[:])
            a0v = cp.tile([96, 1], FP, tag="a0v"); nc.sync.dma_start(a0v[:], a0_d[:])
            cv = [cp.tile([128, 1], FP, tag=f"cv{j}", name=f"cv{j}") for j in range(4)]
            hv = [cp.tile([128, 1], FP, tag=f"hv{j}", name=f"hv{j}") for j in range(4)]
            av = [cp.tile([128, 1], FP, tag=f"av{j}", name=f"av{j}") for j in range(4)]
            for j in range(4):
                nc.sync.dma_start(cv[j][:], cv_d[j])
                nc.sync.dma_start(hv[j][:], hv_d[j])
                nc.sync.dma_start(av[j][:], av_d[j])
            def batch_head(b):
                """Load + replicate raw x, f0 features + silu base (per batch)."""
                xr = xp.tile([96, HW], FP, tag="xr")
                for r in range(8):
                    nc.sync.dma_start(xr[12*r:12*(r+1), :], x_d[b])
                f0t = f0p.tile([108, HW], BF, tag="f0")
                for hh in range(2):
                    hcols = bass.ts(hh, HW // 2)
                    nc.vector._custom_dve(qop, out=f0t[0:96, hcols],
                                          in0=xr[:, hcols], in1=a0v[:],
                                          s0=c0v[:], s1=h0v[:], imm2=B_DEV)
                nc.scalar.activation(f0t[96:108, :], xr[0:12, :], AFT.Silu)
                return f0t

            def stage_A(f0t, hb):
                """L0 matmuls; evacuate h1 (gpsimd) + silu (scalar)."""
                s1 = hp.tile([128, NHB], FP, tag="s1")
                sil1 = hp.tile([64, NHB], BF, tag="sil1")
                for t in range(4):
                    bcols = bass.ts(4*hb + t, NT)
                    lcols = bass.ts(t, NT)
                    ps1 = pp.tile([128, NT], FP, tag="ps1")
                    nc.tensor.matmul(ps1[:], w0[:], f0t[:, bcols],
                                     start=True, stop=True)
                    nc.scalar.activation(s1[:, lcols], ps1[:], AFT.Identity)
                    nc.scalar.activation(sil1[:, lcols], ps1[0:64, :], AFT.Silu)
                return s1, sil1

            def feats(s_t, tag):
                f = [hp.tile([128, NHB], BF, tag=f"{tag}_{j}", name=f"{tag}_{j}")
                     for j in range(4)]
                for j in range(4):
                    nc.vector._custom_dve(qop, out=f[j][:], in0=s_t[:],
                                          in1=av[j][:], s0=cv[j][:],
                                          s1=hv[j][:], imm2=B_DEV)
                return f

            def stage_C(f1, sil1):
                """L1 matmuls; evacuate h2 + silu."""
                s2 = hp.tile([128, NHB], FP, tag="s2")
                sil2 = hp.tile([64, NHB], BF, tag="sil2")
                for t in range(4):
                    lcols = bass.ts(t, NT)
                    ps2 = pp.tile([128, NT], FP, tag="ps2")
                    for j in range(4):
                        nc.tensor.matmul(ps2[:], w1[j][:], f1[j][:, lcols],
                                         start=(j == 0), stop=False)
                    nc.tensor.matmul(ps2[:], b1[:], sil1[:, lcols],
                                     start=False, stop=True)
                    nc.scalar.activation(s2[:, lcols], ps2[:], AFT.Identity)
                    nc.scalar.activation(sil2[:, lcols], ps2[0:64, :], AFT.Silu)
                return s2, sil2

            def stage_E(b, hb, f2, sil2):
                """L2 matmuls + output staging DMA."""
                for t in range(4):
                    bcols = bass.ts(4*hb + t, NT)
                    lcols = bass.ts(t, NT)
                    ps3 = pp.tile([24, NT], FP, tag="ps3")
                    for j in range(4):
                        nc.tensor.matmul(ps3[:], w2[j][:], f2[j][:, lcols],
                                         start=(j == 0), stop=False)
                    nc.tensor.matmul(ps3[:], b2[:], sil2[:, lcols],
                                     start=False, stop=True)
                    yt = hp.tile([24, NT], FP, tag="yt")
                    nc.scalar.activation(yt[:], ps3[:], AFT.Identity)
                    nc.sync.dma_start(out_d[b, :, bcols], yt[:])

            # Two-deep software pipeline.  The DVE queue is in-order, so the
            # f1 stream runs one block ahead of the f2 stream: DVE order is
            # f1(i), f1(i+1), f2(i), f1(i+2), f2(i+1)... — when f1(i) ends,
            # f1(i+1) is already input-ready, and by the time it ends the L1
            # matmuls + h2 evacuation of block i are done so f2(i) starts
            # without a stall.
            blocks = [(b, hb) for b in range(B_PER_CORE) for hb in range(2)]
            f0t = batch_head(0)
            s1_0, sil1_0 = stage_A(f0t, 0)
            pend = [(blocks[0], feats(s1_0, "f1"), sil1_0)]
            for idx in range(len(blocks)):
                if idx + 1 < len(blocks):
                    nb, nhb = blocks[idx + 1]
                    if nhb == 0:
                        f0t = batch_head(nb)
                    s1n, sil1n = stage_A(f0t, nhb)
                    f1n = feats(s1n, "f1")
                    pend.append((blocks[idx + 1], f1n, sil1n))
                (b, hb), f1, sil1 = pend.pop(0)
                s2, sil2 = stage_C(f1, sil1)
                f2 = feats(s2, "f2")
                stage_E(b, hb, f2, sil2)

    nc.compile()
    return nc


# ------------------------------ entry points -------------------------------

def kernel(x, grid0, coef0, sb0, ss0, grid1, coef1, sb1, ss1, grid2, coef2, sb2, ss2):
    if "nc" not in _CACHE:
        _CACHE["nc"] = _build()
    nc = _CACHE["nc"]

    inputs = {"x": x, "coef0": coef0, "sb0": sb0, "ss0": ss0,
              "coef1": coef1, "sb1": sb1, "ss1": ss1,
              "coef2": coef2, "sb2": sb2, "ss2": ss2}
    consts = _host_weights(inputs)

    xf = np.asarray(x, np.float32).reshape(32, 12, HW)
    maps = []
    for c in range(N_CORES):
        m = dict(consts)
        m["x_in"] = np.ascontiguousarray(xf[c*B_PER_CORE:(c+1)*B_PER_CORE])
        maps.append(m)
    res = run_bass_kernel_spmd(nc, maps, core_ids=list(range(N_CORES)))
    _CACHE["maps"] = maps
    out = np.empty((32, 24, HW), np.float32)
    for c in range(N_CORES):
        out[c*B_PER_CORE:(c+1)*B_PER_CORE] = res.results[c]["y_out"]
    return out.reshape(32, 24, 64, 64)


def _install_ntff_hook():
    import sys, types
    if "antenv.axon_hooks" in sys.modules:
        return
    state = {"hook": None}
    mod = types.ModuleType("antenv.axon_hooks")
    mod.set_axon_ntff_profile_hook = lambda h: state.__setitem__("hook", h)
    mod.get_axon_ntff_profile_hook = lambda: state["hook"]
    sys.modules["antenv.axon_hooks"] = mod
    import antenv
    antenv.axon_hooks = mod
    from trn_agent_boot.trn_boot import _ntff_profile_via_ctypes
    hook = _ntff_profile_via_ctypes("/opt/axon/libaxon_pjrt.so")
    if hook is not None:
        mod.set_axon_ntff_profile_hook(hook)


def profile():
    _install_ntff_hook()
    nc = _CACHE["nc"]
    res = run_bass_kernel_spmd(nc, _CACHE["maps"], core_ids=list(range(N_CORES)),
                               trace=True)
    return res.exec_time_ns, getattr(res, "instructions_and_trace", None)


# revision 16
# speedup vs baseline: 1.6658x; 1.0182x over previous
"""Trainium2 Bass kernel for nn_KANCouplingNet (3-layer KAN MLP, 12-64-64-24).

Each KAN layer: y = silu(h) @ sb + B(s) @ W with s = h/0.4 + 5.5 and B the
cubic B-spline basis (8 functions per channel).  The exact cardinal basis
needs >=2 DVE ops per basis value (the M4 two-cube formula exceeds the 8-op
DVE pipeline), so instead the basis is replaced by 8 quartic bump features
per channel:

    psi_k(s) = T^2 (T + a_k)(T + b),   T = relu(h_k - |s - c_k|)

computed by ONE custom DVE instruction (8 ALU ops, per-partition c/h/a via
scalar slots, b via imm2).  The transform features->basis is re-fit on the
host per channel with empirical s-density weighting (mini-forward on a pixel
subsample); end-to-end validation vs the exact reference gives ~4.5e-4
relative error (gate 2e-2), robust to bf16 features/weights (5.3e-4).

This halves DVE work vs the exact two-cube kernel (1 op/basis value) and
nearly halves the matmul count.  Sharding: pure data parallel over batch
(32 images -> 4 per core); features built per half-batch (2048 px) blocks,
software-pipelined one stage ahead so TensorE never stalls on the DVE.
"""
import numpy as np
import ml_dtypes

import concourse.bacc as bacc
import concourse.bass as bass
import concourse.mybir as mybir
import concourse.tile as tile
from concourse.bass_utils import run_bass_kernel_spmd

FP = mybir.dt.float32
BF = mybir.dt.bfloat16
AFT = mybir.ActivationFunctionType

N_CORES = 8
B_PER_CORE = 4
HW = 64 * 64            # 4096 px per image
NT = 512                # matmul moving dim (one PSUM bank)
NHB = 2048              # feature-block pixels (half batch)
S_SCALE = 2.5
S_BIAS = 5.5
K = 8                   # feature rows per input channel

# Generator params (offline fit; end-to-end 4.5e-4 vs exact reference).
A_ROWS = np.array([-0.3615, -0.4926, -0.5196, -0.5232,
                   -0.5232, -0.5196, -0.4926, -0.3615], np.float64)
B_GLOB = -2.5643
C_ROWS = np.arange(8, dtype=np.float64) + 2.0
H_ROWS = np.full(8, 2.0, np.float64)

# psi is scale-invariant: T(s) = S_SCALE * T'(h) with T' built from raw h via
# c' = (c - S_BIAS)/S_SCALE etc., and psi = S_SCALE^4 * psi'.  The device op
# therefore consumes raw x / raw pre-activations (no s staging act), with the
# S_SCALE^4 factor folded into the matmul weights.
C_DEV = (C_ROWS - S_BIAS) / S_SCALE
H_DEV = H_ROWS / S_SCALE
A_DEV = A_ROWS / S_SCALE
B_DEV = B_GLOB / S_SCALE
W_FOLD = S_SCALE ** 4

_CACHE = {}
_QOP = None


def _register_quartic_op():
    """out = T^2 (T + in1)(T + imm2), T = relu(s1 - |in0 - s0|)."""
    global _QOP
    if _QOP is not None:
        return _QOP
    from concourse import dve_ops
    from concourse.dve_spec import (AluOp, Bin, C0, C1, C2, C3, Spec, Src0,
                                    _spill_c3_to_src1, lower, relu, sq)
    from concourse.dve_uop import DveOpSpec

    for op in dve_ops.OPS:
        if op.name == "KANQ_ANT":
            _QOP = op
            return op

    w = Bin(AluOp.ABSOLUTE_DIFF, Src0, C0)
    T = relu(Bin(AluOp.SUBTRACT, C1, w))
    body = _spill_c3_to_src1(sq(T) * (T + C3) * (T + C2))

    def _ref(in0, in1, s0, s1, imm2):
        in0 = np.asarray(in0, np.float32)
        tail = [1] * (in0.ndim - 1)
        c = np.asarray(s0, np.float32).reshape(-1, *tail)
        h = np.asarray(s1, np.float32).reshape(-1, *tail)
        a = np.asarray(in1, np.float32).reshape(-1, *tail)
        T = np.maximum(h - np.abs(in0 - c), 0.0)
        return (T * T * (T + a) * (T + imm2)).astype(np.float32)

    spec = Spec(body=body, reference=_ref)
    row = dve_ops._CUSTOM_DVE_ROW_BASE + len(dve_ops.OPS)
    shas = {}
    for ver in ("v3", "v4"):
        tmp = DveOpSpec(name="KANQ_ANT", opcode=row,
                        uops=lower(spec, ver=ver), rd1_en=True)
        shas[ver] = tmp.sha(ver)
    op = dve_ops.DveOp("KANQ_ANT", spec, subdim=False, uops_sha=shas)
    dve_ops.OPS.append(op)
    dve_ops._SUB_OPCODE_FOR_NAME[op.name] = row
    dve_ops.CUSTOM_DVE_SPECS[op.name] = spec
    _QOP = op
    return op


# --------------------------- host-side math --------------------------------

def _m4(v):
    u = np.abs(v - 2.0)
    r2 = np.maximum(2.0 - u, 0.0)
    r1 = np.maximum(1.0 - u, 0.0)
    return (r2**3 - 4.0 * r1**3) / 6.0


def _silu(x):
    return x / (1.0 + np.exp(-x))


def _psi(s):
    """s: (N,) -> (N, 8) quartic bump features."""
    T = np.maximum(H_ROWS - np.abs(s[:, None] - C_ROWS), 0.0)
    return T * T * (T + A_ROWS) * (T + B_GLOB)


def _fit_layer(coef, ss, s_samp):
    """Per-channel density-weighted lstsq: features -> spline weights.

    coef: (din, dout, 8); ss: (din, dout); s_samp: (n, din)
    returns Wfeat (din, 8, dout) float64
    """
    din, dout, _ = coef.shape
    sgrid = np.linspace(-3.0, 14.0, 1201)
    Psi = _psi(sgrid)                                   # (S, 8)
    Mtgt = np.stack([_m4(sgrid - g) for g in range(8)], 1)
    Wout = np.zeros((din, K, dout))
    for i in range(din):
        hist, edges = np.histogram(s_samp[:, i], bins=120,
                                   range=(-3.0, 14.0), density=True)
        centers = 0.5 * (edges[:-1] + edges[1:])
        wt = np.interp(sgrid, centers, hist) + 1e-3
        sw = np.sqrt(wt)[:, None]
        C, *_ = np.linalg.lstsq(sw * Psi, sw * Mtgt, rcond=None)  # (8, 8)
        Wout[i] = C @ (coef[i] * ss[i][:, None]).T                # (8, dout)
    return Wout


def _host_weights(inputs):
    """Mini-forward for s-samples + per-layer fits; assemble device arrays."""
    x = np.asarray(inputs["x"], np.float64)
    hs = np.transpose(x, (0, 2, 3, 1)).reshape(-1, 12)
    rng = np.random.default_rng(0)
    samp = hs[rng.choice(hs.shape[0], 16384, replace=False)]

    Ws = []
    h = samp
    for li in range(3):
        coef = np.asarray(inputs[f"coef{li}"], np.float64)
        sb = np.asarray(inputs[f"sb{li}"], np.float64)
        ss = np.asarray(inputs[f"ss{li}"], np.float64)
        s = S_SCALE * h + S_BIAS
        Ws.append(_fit_layer(coef, ss, s))
        # exact forward for next layer's sample distribution
        Bsp = np.stack([_m4(s - g) for g in range(8)], -1)       # (n, din, 8)
        h = _silu(h) @ sb + np.einsum('nig,iog->no', Bsp, coef * ss[:, :, None])

    bf = ml_dtypes.bfloat16
    sb0 = np.asarray(inputs["sb0"], np.float64)
    sb1 = np.asarray(inputs["sb1"], np.float64)
    sb2 = np.asarray(inputs["sb2"], np.float64)

    # L0 stationary [108, 128]: rows 0..95 = (k = p//12, i = p%12) features,
    # rows 96..107 = silu base; cols duplicated (o, o+64).
    w0 = np.zeros((108, 128))
    for p in range(96):
        k, i = p // 12, p % 12
        w0[p, 0:64] = W_FOLD * Ws[0][i, k]
        w0[p, 64:128] = W_FOLD * Ws[0][i, k]
    w0[96:108, 0:64] = sb0
    w0[96:108, 64:128] = sb0

    # mid stationary per page j: [128, mcols]; row p: ch=p%64, k=j+4*(p//64)
    def midw(W, dout, dup):
        mc = 128 if dup else dout
        out = np.zeros((4, 128, mc))
        for j in range(4):
            for grp in range(2):
                k = j + 4 * grp
                blk = W_FOLD * W[:, k, :]              # (64, dout)
                out[j, 64*grp:64*grp+64, 0:dout] = blk
                if dup:
                    out[j, 64*grp:64*grp+64, 64:128] = blk
        return out

    w1 = midw(Ws[1], 64, True)
    w2 = midw(Ws[2], 24, False)
    b1 = np.zeros((64, 128)); b1[:, 0:64] = sb1; b1[:, 64:128] = sb1
    b2 = sb2

    # DVE per-partition scalars (raw-input scale)
    c0v = np.zeros((96, 1), np.float32); h0v = np.zeros((96, 1), np.float32)
    a0v = np.zeros((96, 1), np.float32)
    for p in range(96):
        k = p // 12
        c0v[p], h0v[p], a0v[p] = C_DEV[k], H_DEV[k], A_DEV[k]
    cv = np.zeros((4, 128, 1), np.float32); hv = np.zeros((4, 128, 1), np.float32)
    av = np.zeros((4, 128, 1), np.float32)
    for j in range(4):
        for p in range(128):
            k = j + 4 * (p // 64)
            cv[j, p], hv[j, p], av[j, p] = C_DEV[k], H_DEV[k], A_DEV[k]

    return {
        "w0": w0.astype(bf), "w1": w1.astype(bf), "b1": b1.astype(bf),
        "w2": w2.astype(bf), "b2": b2.astype(bf),
        "c0v": c0v, "h0v": h0v, "a0v": a0v,
        "cv": cv, "hv": hv, "av": av,
    }


# --------------------------- device program --------------------------------

def _build():
    qop = _register_quartic_op()
    nc = bacc.Bacc("TRN2", target_bir_lowering=False, debug=False,
                   enable_asserts=False, num_devices=N_CORES)

    x_d = nc.dram_tensor("x_in", [B_PER_CORE, 12, HW], FP, kind="ExternalInput").ap()
    out_d = nc.dram_tensor("y_out", [B_PER_CORE, 24, HW], FP, kind="ExternalOutput").ap()
    w0_d = nc.dram_tensor("w0", [108, 128], BF, kind="ExternalInput").ap()
    w1_d = nc.dram_tensor("w1", [4, 128, 128], BF, kind="ExternalInput").ap()
    b1_d = nc.dram_tensor("b1", [64, 128], BF, kind="ExternalInput").ap()
    w2_d = nc.dram_tensor("w2", [4, 128, 24], BF, kind="ExternalInput").ap()
    b2_d = nc.dram_tensor("b2", [64, 24], BF, kind="ExternalInput").ap()
    c0_d = nc.dram_tensor("c0v", [96, 1], FP, kind="ExternalInput").ap()
    h0_d = nc.dram_tensor("h0v", [96, 1], FP, kind="ExternalInput").ap()
    a0_d = nc.dram_tensor("a0v", [96, 1], FP, kind="ExternalInput").ap()
    cv_d = nc.dram_tensor("cv", [4, 128, 1], FP, kind="ExternalInput").ap()
    hv_d = nc.dram_tensor("hv", [4, 128, 1], FP, kind="ExternalInput").ap()
    av_d = nc.dram_tensor("av", [4, 128, 1], FP, kind="ExternalInput").ap()

    with tile.TileContext(nc) as tc:
        with (
            tc.tile_pool(name="consts", bufs=1) as cp,
            tc.tile_pool(name="xr", bufs=2) as xp,
            tc.tile_pool(name="f0", bufs=2) as f0p,
            tc.tile_pool(name="hb", bufs=2) as hp,
            tc.tile_pool(name="ps", bufs=2, space="PSUM") as pp,
        ):
            w0 = cp.tile([108, 128], BF, tag="w0")
            nc.sync.dma_start(w0[:], w0_d[:])
            w1 = [cp.tile([128, 128], BF, tag=f"w1_{j}", name=f"w1_{j}") for j in range(4)]
            w2 = [cp.tile([128, 24], BF, tag=f"w2_{j}", name=f"w2_{j}") for j in range(4)]
            for j in range(4):
                nc.sync.dma_start(w1[j][:], w1_d[j])
                nc.sync.dma_start(w2[j][:], w2_d[j])
            b1 = cp.tile([64, 128], BF, tag="b1")
            nc.sync.dma_start(b1[:], b1_d[:])
            b2 = cp.tile([64, 24], BF, tag="b2")
            nc.sync.dma_start(b2[:], b2_d[:])
            c0v = cp.tile([96, 1], FP, tag="c0v"); nc.sync.dma_start(c0v[:], c0_d[:])
            h0v = cp.tile([96, 1], FP, tag="h0v"); nc.sync.dma_start(h0v[:], h0_d[:])
            a0v = cp.tile([96, 1], FP, tag="a0v"); nc.sync.dma_start(a0v[:], a0_d[:])
            cv = [cp.tile([128, 1], FP, tag=f"cv{j}", name=f"cv{j}") for j in range(4)]
            hv = [cp.tile([128, 1], FP, tag=f"hv{j}", name=f"hv{j}") for j in range(4)]
            av = [cp.tile([128, 1], FP, tag=f"av{j}", name=f"av{j}") for j in range(4)]
            for j in range(4):
                nc.sync.dma_start(cv[j][:], cv_d[j])
                nc.sync.dma_start(hv[j][:], hv_d[j])
                nc.sync.dma_start(av[j][:], av_d[j])
            def batch_head(b):
                """Load + replicate raw x, f0 features + silu base (per batch)."""
                xr = xp.tile([96, HW], FP, tag="xr")
                for r in range(8):
                    nc.sync.dma_start(xr[12*r:12*(r+1), :], x_d[b])
                f0t = f0p.tile([108, HW], BF, tag="f0")
                for hh in range(2):
                    hcols = bass.ts(hh, HW // 2)
                    nc.vector._custom_dve(qop, out=f0t[0:96, hcols],
                                          in0=xr[:, hcols], in1=a0v[:],
                                          s0=c0v[:], s1=h0v[:], imm2=B_DEV)
                nc.scalar.activation(f0t[96:108, :], xr[0:12, :], AFT.Silu)
                return f0t

            def stage_A(f0t, hb):
                """L0 matmuls; evacuate h1 (gpsimd) + silu (scalar)."""
                s1 = hp.tile([128, NHB], FP, tag="s1")
                sil1 = hp.tile([64, NHB], BF, tag="sil1")
                for t in range(4):
                    bcols = bass.ts(4*hb + t, NT)
                    lcols = bass.ts(t, NT)
                    ps1 = pp.tile([128, NT], FP, tag="ps1")
                    nc.tensor.matmul(ps1[:], w0[:], f0t[:, bcols],
                                     start=True, stop=True)
                    nc.scalar.activation(s1[:, lcols], ps1[:], AFT.Identity)
                    nc.scalar.activation(sil1[:, lcols], ps1[0:64, :], AFT.Silu)
                return s1, sil1

            def feats(s_t, tag):
                f = [hp.tile([128, NHB], BF, tag=f"{tag}_{j}", name=f"{tag}_{j}")
                     for j in range(4)]
                for j in range(4):
                    nc.vector._custom_dve(qop, out=f[j][:], in0=s_t[:],
                                          in1=av[j][:], s0=cv[j][:],
                                          s1=hv[j][:], imm2=B_DEV)
                return f

            def stage_C(f1, sil1):
                """L1 matmuls; evacuate h2 + silu."""
                s2 = hp.tile([128, NHB], FP, tag="s2")
                sil2 = hp.tile([64, NHB], BF, tag="sil2")
                for t in range(4):
                    lcols = bass.ts(t, NT)
                    ps2 = pp.tile([128, NT], FP, tag="ps2")
                    for j in range(4):
                        nc.tensor.matmul(ps2[:], w1[j][:], f1[j][:, lcols],
                                         start=(j == 0), stop=False)
                    nc.tensor.matmul(ps2[:], b1[:], sil1[:, lcols],
                                     start=False, stop=True)
                    nc.scalar.activation(s2[:, lcols], ps2[:], AFT.Identity)
                    nc.scalar.activation(sil2[:, lcols], ps2[0:64, :], AFT.Silu)
                return s2, sil2

            def stage_E(b, hb, f2, sil2):
                """L2 matmuls + output staging DMA."""
                for t in range(4):
                    bcols = bass.ts(4*hb + t, NT)
                    lcols = bass.ts(t, NT)
                    ps3 = pp.tile([24, NT], FP, tag="ps3")
                    for j in range(4):
                        nc.tensor.matmul(ps3[:], w2[j][:], f2[j][:, lcols],
                                         start=(j == 0), stop=False)
                    nc.tensor.matmul(ps3[:], b2[:], sil2[:, lcols],
                                     start=False, stop=True)
                    yt = hp.tile([24, NT], FP, tag="yt")
                    nc.scalar.activation(yt[:], ps3[:], AFT.Identity)
                    nc.sync.dma_start(out_d[b, :, bcols], yt[:])

            # Two-deep software pipeline.  The DVE queue is in-order, so the
            # f1 stream runs one block ahead of the f2 stream: DVE order is
            # f1(i), f1(i+1), f2(i), f1(i+2), f2(i+1)... — when f1(i) ends,
            # f1(i+1) is already input-ready, and by the time it ends the L1
            # matmuls + h2 evacuation of block i are done so f2(i) starts
            # without a stall.
            blocks = [(b, hb) for b in range(B_PER_CORE) for hb in range(2)]
            f0t_cur = batch_head(0)
            f0t_nxt = None
            s1_0, sil1_0 = stage_A(f0t_cur, 0)
            pend = [(blocks[0], feats(s1_0, "f1"), sil1_0)]
            for idx in range(len(blocks)):
                # prefetch the next batch's x-replication DMAs + f0 features
                # two blocks ahead so the DVE never waits on the DMA latency
                if idx % 2 == 0 and idx // 2 + 1 < B_PER_CORE:
                    f0t_nxt = batch_head(idx // 2 + 1)
                if idx + 1 < len(blocks):
                    nb, nhb = blocks[idx + 1]
                    if nhb == 0:
                        f0t_cur = f0t_nxt
                    s1n, sil1n = stage_A(f0t_cur, nhb)
                    f1n = feats(s1n, "f1")
                    pend.append((blocks[idx + 1], f1n, sil1n))
                (b, hb), f1, sil1 = pend.pop(0)
                s2, sil2 = stage_C(f1, sil1)
                f2 = feats(s2, "f2")
                stage_E(b, hb, f2, sil2)

    nc.compile()
    return nc


# ------------------------------ entry points -------------------------------

def kernel(x, grid0, coef0, sb0, ss0, grid1, coef1, sb1, ss1, grid2, coef2, sb2, ss2):
    if "nc" not in _CACHE:
        _CACHE["nc"] = _build()
    nc = _CACHE["nc"]

    inputs = {"x": x, "coef0": coef0, "sb0": sb0, "ss0": ss0,
              "coef1": coef1, "sb1": sb1, "ss1": ss1,
              "coef2": coef2, "sb2": sb2, "ss2": ss2}
    consts = _host_weights(inputs)

    xf = np.asarray(x, np.float32).reshape(32, 12, HW)
    maps = []
    for c in range(N_CORES):
        m = dict(consts)
        m["x_in"] = np.ascontiguousarray(xf[c*B_PER_CORE:(c+1)*B_PER_CORE])
        maps.append(m)
    res = run_bass_kernel_spmd(nc, maps, core_ids=list(range(N_CORES)))
    _CACHE["maps"] = maps
    out = np.empty((32, 24, HW), np.float32)
    for c in range(N_CORES):
        out[c*B_PER_CORE:(c+1)*B_PER_CORE] = res.results[c]["y_out"]
    return out.reshape(32, 24, 64, 64)


def _install_ntff_hook():
    import sys, types
    if "antenv.axon_hooks" in sys.modules:
        return
    state = {"hook": None}
    mod = types.ModuleType("antenv.axon_hooks")
    mod.set_axon_ntff_profile_hook = lambda h: state.__setitem__("hook", h)
    mod.get_axon_ntff_profile_hook = lambda: state["hook"]
    sys.modules["antenv.axon_hooks"] = mod
    import antenv
    antenv.axon_hooks = mod
    from trn_agent_boot.trn_boot import _ntff_profile_via_ctypes
    hook = _ntff_profile_via_ctypes("/opt/axon/libaxon_pjrt.so")
    if hook is not None:
        mod.set_axon_ntff_profile_hook(hook)


def profile():
    _install_ntff_hook()
    nc = _CACHE["nc"]
    res = run_bass_kernel_spmd(nc, _CACHE["maps"], core_ids=list(range(N_CORES)),
                               trace=True)
    return res.exec_time_ns, getattr(res, "instructions_and_trace", None)
